# revision 21
# baseline (speedup 1.0000x reference)
"""Trainium2 Bass kernel for a post-LN transformer encoder layer.

Reference computation (fp32):
    q,k,v = x@Wq+bq, x@Wk+bk, x@Wv+bv        (per-head views, H=16, dk=64)
    attn  = softmax(q k^T / sqrt(dk))         -> returned as output #2
    ctx   = attn @ v
    a_in  = x + ctx@Wo + bo ;  attn_out = LN1(a_in)
    ff    = relu(attn_out@W1 + b1)@W2 + b2
    enc   = LN2(attn_out + ff)                -> returned as output #1
    LN(y) = alpha * (y-mean)/(std+eps) + beta,  std Bessel-corrected (N-1)

Sharding over 8 NeuronCores (one program, per-core data via in_maps):
  - Attention head-parallel: core c owns heads {2c, 2c+1} for BOTH batches
    (columns 128c:128c+128 of Wq/Wk/Wv, rows 128c:128c+128 of ctx^T).
  - After attention an AllToAll re-shards from head-split to token-split
    (2 MiB per core), so the out-proj sum-over-heads happens inside the
    Wo matmul contraction on each core's 512-token chunk.
  - FFN + both LayerNorms token-parallel: core c owns global tokens
    [512c, 512c+512) with full W1/W2 (weights streamed from HBM).

Attention internals per (batch, head-pair), everything fp32:
  PASS-K (k-major): scores^T tiles [128k x 512q] via packed 2-head matmuls
    (K=64 row-tiling at partitions 0/64), exp on ACT (scale=1/8 folded in),
    ctx^T accumulated on PE with a ones-augmented V so the softmax
    denominator falls out of row 64 for free.  ctx is normalized before the
    AllToAll (divide-by-rowsum must happen before the head-sum in Wo) on
    DVE with a partition-broadcast reciprocal tile.
  PASS-Q (q-major): recompute scores [128q x 512k], exp with accum_out
    giving the row-sum per q on the fly, normalize per-partition on DVE,
    DMA 1 MiB tiles straight into the attn output.  Traced alongside the
    Wo phase so it overlaps the AllToAll + out-proj on the hardware.
"""

import numpy as np
from contextlib import ExitStack

import concourse.bass as bass
import concourse.tile as tile
from concourse import bacc, mybir
from concourse.bass_utils import run_bass_kernel_spmd
from concourse.masks import make_identity

FP32 = mybir.dt.float32
AF = mybir.ActivationFunctionType
ALU = mybir.AluOpType
AX = mybir.AxisListType

D = 1024          # d_model
H = 16            # heads
DK = 64           # head dim
F = 4096          # d_ff
B = 2
S = 2048
NCORES = 8
HPC = H // NCORES  # heads per core = 2
T = B * S          # 4096 global tokens
TC = T // NCORES   # 512 tokens per core
LN_SCALE = float(D) / float(D - 1)  # Bessel correction folded into ln(var)


def build_program():
    nc = bacc.Bacc(
        "TRN2",
        target_bir_lowering=False,
        debug=False,
        enable_asserts=False,
        num_devices=NCORES,
    )

    # ---- I/O ----------------------------------------------------------
    x_d = nc.dram_tensor("x", [T, D], FP32, kind="ExternalInput").ap()
    xc_d = nc.dram_tensor("xc", [TC, D], FP32, kind="ExternalInput").ap()
    wq_d = nc.dram_tensor("wq", [D, 128], FP32, kind="ExternalInput").ap()
    wk_d = nc.dram_tensor("wk", [D, 128], FP32, kind="ExternalInput").ap()
    wv_d = nc.dram_tensor("wv", [D, 128], FP32, kind="ExternalInput").ap()
    bq_d = nc.dram_tensor("bq", [1, 128], FP32, kind="ExternalInput").ap()
    bk_d = nc.dram_tensor("bk", [1, 128], FP32, kind="ExternalInput").ap()
    bv_d = nc.dram_tensor("bv", [1, 128], FP32, kind="ExternalInput").ap()
    wo_d = nc.dram_tensor("wo", [D, D], FP32, kind="ExternalInput").ap()
    bo_d = nc.dram_tensor("bo", [1, D], FP32, kind="ExternalInput").ap()
    w1_d = nc.dram_tensor("w1", [D, F], FP32, kind="ExternalInput").ap()
    b1_d = nc.dram_tensor("b1", [1, F], FP32, kind="ExternalInput").ap()
    w2_d = nc.dram_tensor("w2", [F, D], FP32, kind="ExternalInput").ap()
    b2_d = nc.dram_tensor("b2", [1, D], FP32, kind="ExternalInput").ap()
    g1_d = nc.dram_tensor("g1", [1, D], FP32, kind="ExternalInput").ap()
    be1_d = nc.dram_tensor("be1", [1, D], FP32, kind="ExternalInput").ap()
    g2_d = nc.dram_tensor("g2", [1, D], FP32, kind="ExternalInput").ap()
    be2_d = nc.dram_tensor("be2", [1, D], FP32, kind="ExternalInput").ap()
    attn_d = nc.dram_tensor("attn_o", [B, HPC, S, S], FP32, kind="ExternalOutput").ap()
    enc_d = nc.dram_tensor("enc_o", [TC, D], FP32, kind="ExternalOutput").ap()

    def bcast(ap_1n):
        # [1, N] AP -> partition-broadcast AP [[0,128], free dims...]
        return bass.AP(tensor=ap_1n.tensor, offset=ap_1n.offset,
                       ap=[[0, 128]] + list(ap_1n.ap[1:]))

    with tile.TileContext(nc) as tc, ExitStack() as top:
        # Pool lifetimes must nest LIFO per (space, side).  Long-lived pools
        # go on the "right" side, phase-scoped pools on the "left".
        const = top.enter_context(tc.tile_pool(name="const", bufs=1, side="right"))
        identity = const.tile([128, 128], FP32)
        make_identity(nc, identity)
        ones = const.tile([1, 512], FP32)
        nc.vector.memset(ones, 1.0)
        bq_sb = const.tile([1, 128], FP32)
        bk_sb = const.tile([1, 128], FP32)
        bv_sb = const.tile([1, 128], FP32)
        nc.sync.dma_start(out=bq_sb, in_=bq_d)
        nc.sync.dma_start(out=bk_sb, in_=bk_d)
        nc.sync.dma_start(out=bv_sb, in_=bv_d)
        aout = const.tile([128, 4, D], FP32)  # attn_out = LN1(...), lives to the end
        dram = top.enter_context(tc.tile_pool(name="dram", bufs=1, space="DRAM",
                                              side="right"))
        a2a_in = dram.tile([NCORES, 128, 512], FP32)
        a2a_out = dram.tile([NCORES, 128, 512], FP32)
        rsum_d = dram.tile([2, T], FP32)  # DRAM bounce for partition-broadcast

        # persistent attention tensors, two lifetime classes
        pk_stack = ExitStack()           # qT/kT: needed through PASS-Q
        cv_stack = ExitStack()           # v/ctxT/rsum: dead after AllToAll send
        pqk = pk_stack.enter_context(tc.tile_pool(name="pqk", bufs=1, side="right"))
        pcv = cv_stack.enter_context(tc.tile_pool(name="pcv", bufs=1, side="right"))
        qT = pqk.tile([128, T], FP32)        # rows 0:64 head0, 64:128 head1
        kT = pqk.tile([128, T], FP32)
        v0 = pcv.tile([128, 16, 130], FP32)  # batch 0: [tok%128, ktile, 65*2]
        v1 = pcv.tile([128, 16, 130], FP32)  # cols 0:64 h0 |64 ones| 65:129 h1 |129 ones
        v_b = [v0, v1]
        ctxT0 = pcv.tile([65, T], FP32)      # head0 unnormalized ctx^T; row 64 = rowsum
        ctxT1 = pcv.tile([65, T], FP32)      # head1
        for vb in v_b:
            nc.vector.memset(vb[:, :, 64:65], 1.0)
            nc.vector.memset(vb[:, :, 129:130], 1.0)

        # ---- Phase B: transpose x + QKV projections -------------------
        with ExitStack() as pb:
            pbs = pb.enter_context(tc.tile_pool(name="projs", bufs=1))
            wq_sb = pbs.tile([128, 8, 128], FP32)
            wk_sb = pbs.tile([128, 8, 128], FP32)
            wv_sb = pbs.tile([128, 8, 128], FP32)
            nc.sync.dma_start(out=wq_sb, in_=wq_d.rearrange("(a p) c -> p a c", p=128))
            nc.sync.dma_start(out=wk_sb, in_=wk_d.rearrange("(a p) c -> p a c", p=128))
            nc.sync.dma_start(out=wv_sb, in_=wv_d.rearrange("(a p) c -> p a c", p=128))

            xin_p = pb.enter_context(tc.tile_pool(name="xin", bufs=6))
            xt_p = pb.enter_context(tc.tile_pool(name="xt", bufs=2))
            ps_p = pb.enter_context(tc.tile_pool(name="proj_ps", bufs=1, space="PSUM"))

            for tch in range(8):          # token chunks of 512
                bb = tch // 4
                xT = xt_p.tile([128, 8, 512], FP32)  # [d%128, dslab, tok]
                for sub in range(4):
                    x_in = xin_p.tile([128, D], FP32)
                    nc.sync.dma_start(
                        out=x_in,
                        in_=x_d[tch * 512 + sub * 128: tch * 512 + (sub + 1) * 128, :])
                    for dsl in range(8):
                        tr = ps_p.tile([128, 128], FP32, tag="tr", bufs=2)
                        nc.tensor.transpose(tr, x_in[:, dsl * 128:(dsl + 1) * 128], identity)
                        nc.vector.tensor_copy(out=xT[:, dsl, sub * 128:(sub + 1) * 128], in_=tr)
                # q^T / k^T for this chunk: [128(2 heads*dk), 512]
                for w_sb, b_sb, dst in ((wq_sb, bq_sb, qT), (wk_sb, bk_sb, kT)):
                    ps = ps_p.tile([128, 512], FP32, tag="qk", bufs=3)
                    for dsl in range(8):
                        nc.tensor.matmul(ps, w_sb[:, dsl, :], xT[:, dsl, :],
                                         start=(dsl == 0), stop=False)
                    nc.tensor.matmul(ps, b_sb, ones[:, 0:512], start=False, stop=True)
                    nc.vector.tensor_copy(out=dst[:, tch * 512:(tch + 1) * 512], in_=ps)
                # v token-major [128 tok, 128(2 heads)]
                for sub in range(4):
                    kt = (tch % 4) * 4 + sub
                    vps = ps_p.tile([128, 128], FP32, tag="v", bufs=2)
                    for dsl in range(8):
                        nc.tensor.matmul(vps, xT[:, dsl, sub * 128:(sub + 1) * 128],
                                         wv_sb[:, dsl, :], start=(dsl == 0), stop=False)
                    nc.tensor.matmul(vps, ones[:, 0:128], bv_sb, start=False, stop=True)
                    nc.vector.tensor_copy(out=v_b[bb][:, kt, 0:64], in_=vps[:, 0:64])
                    nc.vector.tensor_copy(out=v_b[bb][:, kt, 65:129], in_=vps[:, 64:128])

        # ---- Phase C1 (PASS-K): ctx^T + softmax denominators ----------
        with ExitStack() as pk:
            e_p = pk.enter_context(tc.tile_pool(name="epool", bufs=1))
            sc_p = pk.enter_context(tc.tile_pool(name="sc_ps", bufs=1, space="PSUM"))
            cx_p = pk.enter_context(tc.tile_pool(name="cx_ps", bufs=1, space="PSUM"))
            for bb in range(2):
                tok0 = bb * S
                for qc in range(4):       # 512-wide query chunks
                    q_sl = slice(tok0 + qc * 512, tok0 + (qc + 1) * 512)
                    cxa = cx_p.tile([65, 512], FP32, tag="cxa", bufs=2)
                    cxb = cx_p.tile([65, 512], FP32, tag="cxb", bufs=2)
                    for kt in range(16):  # 128-wide key tiles
                        k_sl = slice(tok0 + kt * 128, tok0 + (kt + 1) * 128)
                        sca = sc_p.tile([128, 512], FP32, tag="sca", bufs=2)
                        scb = sc_p.tile([128, 512], FP32, tag="scb", bufs=2)
                        nc.tensor.matmul(sca, kT[0:64, k_sl], qT[0:64, q_sl],
                                         start=True, stop=True)
                        nc.tensor.matmul(scb, kT[64:128, k_sl], qT[64:128, q_sl],
                                         start=True, stop=True)
                        ea = e_p.tile([128, 512], FP32, tag="ea", bufs=3)
                        eb = e_p.tile([128, 512], FP32, tag="eb", bufs=3)
                        nc.scalar.activation(ea, sca, AF.Exp, scale=0.125)
                        nc.scalar.activation(eb, scb, AF.Exp, scale=0.125)
                        nc.tensor.matmul(cxa, v_b[bb][:, kt, 0:65], ea,
                                         start=(kt == 0), stop=(kt == 15))
                        nc.tensor.matmul(cxb, v_b[bb][:, kt, 65:130], eb,
                                         start=(kt == 0), stop=(kt == 15))
                    nc.vector.tensor_copy(out=ctxT0[:, q_sl], in_=cxa)
                    nc.vector.tensor_copy(out=ctxT1[:, q_sl], in_=cxb)

        # ---- Phase D: normalize ctx^T, AllToAll to token-split --------
        with ExitStack() as pd:
            rec_p = pd.enter_context(tc.tile_pool(name="recip", bufs=1))
            nc.sync.dma_start(out=rsum_d[0:1], in_=ctxT0[64:65, :])
            nc.sync.dma_start(out=rsum_d[1:2], in_=ctxT1[64:65, :])
            for hh, ct in ((0, ctxT0), (1, ctxT1)):
                rec = rec_p.tile([64, T], FP32, name=f"rec{hh}", tag=f"rec{hh}")
                nc.gpsimd.dma_start(
                    out=rec, in_=bcast(rsum_d[hh:hh + 1, :])[0:64])
                nc.vector.reciprocal(out=rec, in_=rec)
                nc.vector.tensor_mul(out=ct[0:64, :], in0=ct[0:64, :], in1=rec)
        for j in range(NCORES):
            nc.sync.dma_start(out=a2a_in[j, 0:64, :],
                              in_=ctxT0[0:64, j * 512:(j + 1) * 512])
            nc.sync.dma_start(out=a2a_in[j, 64:128, :],
                              in_=ctxT1[0:64, j * 512:(j + 1) * 512])
        nc.gpsimd.collective_compute(
            "AllToAll", ALU.bypass,
            replica_groups=[list(range(NCORES))],
            ins=[a2a_in.opt()], outs=[a2a_out.opt()])
        cv_stack.close()

        # ---- Phase C2 (PASS-Q): normalized attention probs ------------
        # Traced before/alongside phase E so the two overlap at runtime.
        c2_stack = ExitStack()
        st_p = c2_stack.enter_context(tc.tile_pool(name="stage", bufs=1))
        sm_p = c2_stack.enter_context(tc.tile_pool(name="smalls", bufs=1))
        sq_p = c2_stack.enter_context(tc.tile_pool(name="sq_ps", bufs=1, space="PSUM"))
        for bb in range(2):
            tok0 = bb * S
            for qt in range(16):      # 128-wide query tiles
                q_sl = slice(tok0 + qt * 128, tok0 + (qt + 1) * 128)
                stg = [st_p.tile([128, S], FP32, name=f"stg{j}",
                                 tag=f"stg{j}", bufs=2) for j in range(2)]
                parts = sm_p.tile([128, 2, 4], FP32, bufs=2)
                for kc in range(4):   # 512-wide key chunks
                    k_sl = slice(tok0 + kc * 512, tok0 + (kc + 1) * 512)
                    for j, base in ((0, slice(0, 64)), (1, slice(64, 128))):
                        sq = sq_p.tile([128, 512], FP32, name=f"sq{j}",
                                       tag=f"sq{j}", bufs=2)
                        nc.tensor.matmul(sq, qT[base, q_sl], kT[base, k_sl],
                                         start=True, stop=True)
                        nc.scalar.activation(
                            stg[j][:, kc * 512:(kc + 1) * 512], sq, AF.Exp,
                            scale=0.125, accum_out=parts[:, j, kc:kc + 1])
                for j in range(2):
                    rq = sm_p.tile([128, 1], FP32, tag="rq", bufs=4)
                    nc.vector.reduce_sum(out=rq, in_=parts[:, j, :], axis=AX.X)
                    nc.vector.reciprocal(out=rq, in_=rq)
                    nc.vector.tensor_scalar_mul(out=stg[j], in0=stg[j], scalar1=rq)
                    nc.sync.dma_start(
                        out=attn_d[bb, j, qt * 128:(qt + 1) * 128, :], in_=stg[j])

        # ---- Phase E: out-proj + residual + LN1 -----------------------
        def layer_norm(pool, src, dst, g_sb, be_sb):
            # src/dst [128, D]; dst = g * (src-mean)/std' + be
            # std' = sqrt(sum c^2/(D-1)) computed as exp(-0.5*ln(var*D/(D-1)))
            # (the reference's +eps=1e-6 on std is dropped: |rel err| ~1e-6)
            stats = pool.tile([128, 2, 6], FP32, tag="stats", bufs=2)
            mv = pool.tile([128, 2], FP32, tag="mv", bufs=2)
            inv = pool.tile([128, 1], FP32, tag="inv", bufs=2)
            nc.vector.bn_stats(out=stats[:, 0, :], in_=src[:, 0:512])
            nc.vector.bn_stats(out=stats[:, 1, :], in_=src[:, 512:1024])
            nc.vector.bn_aggr(out=mv, in_=stats)
            nc.scalar.activation(inv, mv[:, 1:2], AF.Ln, scale=LN_SCALE)
            nc.scalar.activation(inv, inv, AF.Exp, scale=-0.5)
            nc.vector.tensor_scalar(out=dst, in0=src, scalar1=mv[:, 0:1],
                                    scalar2=inv, op0=ALU.subtract, op1=ALU.mult)
            nc.vector.tensor_mul(out=dst, in0=dst, in1=g_sb)
            nc.vector.tensor_add(out=dst, in0=dst, in1=be_sb)

        e_stack = ExitStack()
        ctxr_p = e_stack.enter_context(tc.tile_pool(name="ctxr", bufs=1))
        wo_p = e_stack.enter_context(tc.tile_pool(name="wo", bufs=1))
        ln_p = e_stack.enter_context(tc.tile_pool(name="ln1", bufs=1))
        ai_p = e_stack.enter_context(tc.tile_pool(name="attn_in", bufs=2))
        wo_ps = e_stack.enter_context(tc.tile_pool(name="wo_ps", bufs=1, space="PSUM"))
        bo_sb = ln_p.tile([1, D], FP32)
        g1_sb = ln_p.tile([128, D], FP32)
        be1_sb = ln_p.tile([128, D], FP32)
        nc.sync.dma_start(out=bo_sb, in_=bo_d)
        nc.gpsimd.dma_start(out=g1_sb, in_=bcast(g1_d))
        nc.gpsimd.dma_start(out=be1_sb, in_=bcast(be1_d))
        ctxr = ctxr_p.tile([128, 8, 512], FP32)
        for dsl in range(8):
            nc.sync.dma_start(out=ctxr[:, dsl, :], in_=a2a_out[dsl])
        wo_sb = []
        for dsl in range(8):
            w = wo_p.tile([128, D], FP32, name=f"wo{dsl}", tag="wo_slab", bufs=8)
            nc.sync.dma_start(out=w, in_=wo_d[dsl * 128:(dsl + 1) * 128, :])
            wo_sb.append(w)
        for sub in range(4):
            a_in = ai_p.tile([128, D], FP32, tag="a_in", bufs=2)
            xc_sb = ai_p.tile([128, D], FP32, tag="xc_sb", bufs=2)
            nc.sync.dma_start(out=xc_sb, in_=xc_d[sub * 128:(sub + 1) * 128, :])
            for oc in range(2):
                ps = wo_ps.tile([128, 512], FP32, tag="wops", bufs=3)
                for dsl in range(8):
                    nc.tensor.matmul(ps, ctxr[:, dsl, sub * 128:(sub + 1) * 128],
                                     wo_sb[dsl][:, oc * 512:(oc + 1) * 512],
                                     start=(dsl == 0), stop=False)
                nc.tensor.matmul(ps, ones[:, 0:128], bo_sb[:, oc * 512:(oc + 1) * 512],
                                 start=False, stop=True)
                nc.vector.tensor_add(out=a_in[:, oc * 512:(oc + 1) * 512],
                                     in0=ps, in1=xc_sb[:, oc * 512:(oc + 1) * 512])
            layer_norm(ln_p, a_in, aout[:, sub, :], g1_sb, be1_sb)

        e_stack.close()
        c2_stack.close()
        pk_stack.close()

        # ---- Phase F+G: transpose attn_out, fc1 (relu) ----------------
        fg_stack = ExitStack()
        at_p = fg_stack.enter_context(tc.tile_pool(name="at", bufs=1))
        w1_p = fg_stack.enter_context(tc.tile_pool(name="w1", bufs=2))
        ht_p = fg_stack.enter_context(tc.tile_pool(name="ht", bufs=1))
        g_ps = fg_stack.enter_context(tc.tile_pool(name="g_ps", bufs=1, space="PSUM"))
        aT = at_p.tile([128, 8, 512], FP32)
        for sub in range(4):
            for dsl in range(8):
                tr = g_ps.tile([128, 128], FP32, tag="tr2", bufs=2)
                nc.tensor.transpose(tr, aout[:, sub, dsl * 128:(dsl + 1) * 128], identity)
                nc.vector.tensor_copy(out=aT[:, dsl, sub * 128:(sub + 1) * 128], in_=tr)
        hT = ht_p.tile([128, 32, 512], FP32)   # [f%128, fslab, tok]
        b1_sb = at_p.tile([1, F], FP32)
        nc.sync.dma_start(out=b1_sb, in_=b1_d)
        for fsl in range(32):
            # stream W1 column-block [1024, 128] as [128 dpart, 8 dslab, 128 f]
            w1c = w1_p.tile([128, 8, 128], FP32, tag="w1_col", bufs=3)
            nc.sync.dma_start(
                out=w1c,
                in_=w1_d[:, fsl * 128:(fsl + 1) * 128].rearrange(
                    "(a p) c -> p a c", p=128))
            ps = g_ps.tile([128, 512], FP32, tag="hps", bufs=3)
            for dsl in range(8):
                nc.tensor.matmul(ps, w1c[:, dsl, :],
                                 aT[:, dsl, :], start=(dsl == 0), stop=False)
            nc.tensor.matmul(ps, b1_sb[:, fsl * 128:(fsl + 1) * 128],
                             ones[:, 0:512], start=False, stop=True)
            nc.vector.tensor_scalar_max(out=hT[:, fsl, :], in0=ps, scalar1=0.0)
        fg_stack.close()

        # ---- Phase H+I: fc2 + residual + LN2, write enc ---------------
        with ExitStack() as phi:
            w2_p = phi.enter_context(tc.tile_pool(name="w2", bufs=3))
            ln2_p = phi.enter_context(tc.tile_pool(name="ln2", bufs=1))
            enc_p = phi.enter_context(tc.tile_pool(name="encs", bufs=2))
            f_ps = phi.enter_context(tc.tile_pool(name="f_ps", bufs=1, space="PSUM"))
            b2_sb = ln2_p.tile([1, D], FP32)
            g2_sb = ln2_p.tile([128, D], FP32)
            be2_sb = ln2_p.tile([128, D], FP32)
            nc.sync.dma_start(out=b2_sb, in_=b2_d)
            nc.gpsimd.dma_start(out=g2_sb, in_=bcast(g2_d))
            nc.gpsimd.dma_start(out=be2_sb, in_=bcast(be2_d))
            groups = {}
            for sub in range(4):
                for oc in range(2):
                    groups[(sub, oc)] = f_ps.tile(
                        [128, 512], FP32, name=f"ff{sub}{oc}",
                        tag=f"ff{sub}{oc}", bufs=1)
            for fsl in range(32):
                w2s = w2_p.tile([128, D], FP32, tag="w2_slab", bufs=3)
                nc.sync.dma_start(out=w2s, in_=w2_d[fsl * 128:(fsl + 1) * 128, :])
                for sub in range(4):
                    for oc in range(2):
                        nc.tensor.matmul(groups[(sub, oc)],
                                         hT[:, fsl, sub * 128:(sub + 1) * 128],
                                         w2s[:, oc * 512:(oc + 1) * 512],
                                         start=(fsl == 0), stop=False)
            for sub in range(4):
                enc_pre = enc_p.tile([128, D], FP32, tag="encpre", bufs=2)
                enc_out = enc_p.tile([128, D], FP32, tag="encout", bufs=2)
                for oc in range(2):
                    ps = groups[(sub, oc)]
                    nc.tensor.matmul(ps, ones[:, 0:128],
                                     b2_sb[:, oc * 512:(oc + 1) * 512],
                                     start=False, stop=True)
                    nc.vector.tensor_add(out=enc_pre[:, oc * 512:(oc + 1) * 512],
                                         in0=ps, in1=aout[:, sub, oc * 512:(oc + 1) * 512])
                layer_norm(ln2_p, enc_pre, enc_out, g2_sb, be2_sb)
                nc.sync.dma_start(out=enc_d[sub * 128:(sub + 1) * 128, :], in_=enc_out)

    nc.compile()
    return nc


def make_in_maps(inputs):
    inp = {k: np.asarray(v) for k, v in inputs.items()}
    x = np.ascontiguousarray(inp["x"].reshape(T, D).astype(np.float32))
    r1 = lambda a: np.ascontiguousarray(a.astype(np.float32).reshape(1, -1))
    in_maps = []
    for c in range(NCORES):
        cs = slice(128 * c, 128 * (c + 1))
        in_maps.append({
            "x": x,
            "xc": np.ascontiguousarray(x[TC * c: TC * (c + 1)]),
            "wq": np.ascontiguousarray(inp["Wq"].astype(np.float32)[:, cs]),
            "wk": np.ascontiguousarray(inp["Wk"].astype(np.float32)[:, cs]),
            "wv": np.ascontiguousarray(inp["Wv"].astype(np.float32)[:, cs]),
            "bq": r1(inp["bq"][cs]),
            "bk": r1(inp["bk"][cs]),
            "bv": r1(inp["bv"][cs]),
            "wo": np.ascontiguousarray(inp["Wo"].astype(np.float32)),
            "bo": r1(inp["bo"]),
            "w1": np.ascontiguousarray(inp["W1"].astype(np.float32)),
            "b1": r1(inp["b1"]),
            "w2": np.ascontiguousarray(inp["W2"].astype(np.float32)),
            "b2": r1(inp["b2"]),
            "g1": r1(inp["alpha1"]),
            "be1": r1(inp["beta1"]),
            "g2": r1(inp["alpha2"]),
            "be2": r1(inp["beta2"]),
        })
    return in_maps


def assemble(results):
    attn = np.empty((B, H, S, S), np.float32)
    enc = np.empty((T, D), np.float32)
    for c in range(NCORES):
        attn[:, 2 * c: 2 * c + 2] = results[c]["attn_o"]
        enc[TC * c: TC * (c + 1)] = results[c]["enc_o"]
    return enc.reshape(B, S, D), attn


def kernel(**inputs):
    nc = build_program()
    in_maps = make_in_maps(inputs)
    res = run_bass_kernel_spmd(nc, in_maps, list(range(NCORES))).results
    return assemble(res)


# revision 44
# speedup vs baseline: 1.1183x; 1.1183x over previous
"""Trainium2 Bass kernel for a post-LN transformer encoder layer.

Reference computation (fp32):
    q,k,v = x@Wq+bq, x@Wk+bk, x@Wv+bv        (per-head views, H=16, dk=64)
    attn  = softmax(q k^T / sqrt(dk))         -> returned as output #2
    ctx   = attn @ v
    a_in  = x + ctx@Wo + bo ;  attn_out = LN1(a_in)
    ff    = relu(attn_out@W1 + b1)@W2 + b2
    enc   = LN2(attn_out + ff)                -> returned as output #1
    LN(y) = alpha * (y-mean)/(std+eps) + beta,  std Bessel-corrected (N-1)

Sharding over 8 NeuronCores (one program, per-core data via in_maps):
  - Attention head-parallel: core c owns heads {2c, 2c+1} for BOTH batches
    (columns 128c:128c+128 of Wq/Wk/Wv, rows 128c:128c+128 of ctx^T).
  - After attention an AllToAll re-shards from head-split to token-split
    (2 MiB per core), so the out-proj sum-over-heads happens inside the
    Wo matmul contraction on each core's 512-token chunk.
  - FFN + both LayerNorms token-parallel: core c owns global tokens
    [512c, 512c+512) with full W1/W2 (streamed from HBM as bf16).

Precision choices (graded tolerance is ~2e-2 relative):
  - All attention / out-proj matmuls run in float32r (TF32-like 4-byte
    mode, ~1e-5 relative) — full PE rate, fp32 would be 1/4 rate.
  - The FFN runs in bf16 (weights pre-cast host-side, activations cast at
    PSUM eviction) — halves the 32 MiB W1/W2 HBM traffic; ~5e-3 relative
    on the ff term only, which the final LayerNorm keeps well in budget.
  - exp/softmax, LayerNorm stats, residuals, outputs all fp32.

Attention internals per (batch, head-pair):
  PASS-K (k-major): scores^T tiles [128k x 1024q] via packed 2-head matmuls
    (K=64 row-tiling at partitions 0/64), exp on ACT (scale=1/8 folded in),
    ctx^T accumulated on PE with a ones-augmented V so the softmax
    denominator falls out of row 64 for free.  ctx is normalized before the
    AllToAll (divide-by-rowsum must happen before the head-sum in Wo).
  PASS-Q (q-major): recompute scores [128q x 1024k], exp with accum_out
    giving the row-sum per q on the fly, normalize per-partition on DVE,
    DMA 1 MiB tiles straight into the attn output.  Traced alongside the
    Wo/FFN phases so everything overlaps the attn-output DMA stream.
"""

import numpy as np
from contextlib import ExitStack

import ml_dtypes
import concourse.bass as bass
import concourse.tile as tile
from concourse import bacc, mybir
from concourse.bass_utils import run_bass_kernel_spmd
from concourse.masks import make_identity

FP32 = mybir.dt.float32
F32R = mybir.dt.float32r
BF16 = mybir.dt.bfloat16
AF = mybir.ActivationFunctionType
ALU = mybir.AluOpType
AX = mybir.AxisListType

D = 1024          # d_model
H = 16            # heads
DK = 64           # head dim
F = 4096          # d_ff
B = 2
S = 2048
NCORES = 8
HPC = H // NCORES  # heads per core = 2
T = B * S          # 4096 global tokens
TC = T // NCORES   # 512 tokens per core
LN_SCALE = float(D) / float(D - 1)  # Bessel correction folded into ln(var)


def build_program():
    nc = bacc.Bacc(
        "TRN2",
        target_bir_lowering=False,
        debug=False,
        enable_asserts=False,
        num_devices=NCORES,
    )

    # ---- I/O ----------------------------------------------------------
    xt_d = nc.dram_tensor("xt", [D, T], FP32, kind="ExternalInput").ap()
    xc_d = nc.dram_tensor("xc", [TC, D], FP32, kind="ExternalInput").ap()
    wq_d = nc.dram_tensor("wq", [D, 128], FP32, kind="ExternalInput").ap()
    wk_d = nc.dram_tensor("wk", [D, 128], FP32, kind="ExternalInput").ap()
    wv_d = nc.dram_tensor("wv", [D, 128], FP32, kind="ExternalInput").ap()
    bq_d = nc.dram_tensor("bq", [1, 128], FP32, kind="ExternalInput").ap()
    bk_d = nc.dram_tensor("bk", [1, 128], FP32, kind="ExternalInput").ap()
    bv_d = nc.dram_tensor("bv", [1, 128], FP32, kind="ExternalInput").ap()
    wo_d = nc.dram_tensor("wo", [D, D], FP32, kind="ExternalInput").ap()
    bo_d = nc.dram_tensor("bo", [1, D], FP32, kind="ExternalInput").ap()
    w1_d = nc.dram_tensor("w1", [D, F], BF16, kind="ExternalInput").ap()
    b1_d = nc.dram_tensor("b1", [1, F], BF16, kind="ExternalInput").ap()
    w2_d = nc.dram_tensor("w2", [F, D], BF16, kind="ExternalInput").ap()
    b2_d = nc.dram_tensor("b2", [1, D], BF16, kind="ExternalInput").ap()
    g1_d = nc.dram_tensor("g1", [1, D], FP32, kind="ExternalInput").ap()
    be1_d = nc.dram_tensor("be1", [1, D], FP32, kind="ExternalInput").ap()
    g2_d = nc.dram_tensor("g2", [1, D], FP32, kind="ExternalInput").ap()
    be2_d = nc.dram_tensor("be2", [1, D], FP32, kind="ExternalInput").ap()
    attn_d = nc.dram_tensor("attn_o", [B, HPC, S, S], FP32, kind="ExternalOutput").ap()
    enc_d = nc.dram_tensor("enc_o", [TC, D], FP32, kind="ExternalOutput").ap()

    def bcast(ap_1n):
        # [1, N] AP -> partition-broadcast AP [[0,128], free dims...]
        return bass.AP(tensor=ap_1n.tensor, offset=ap_1n.offset,
                       ap=[[0, 128]] + list(ap_1n.ap[1:]))

    def mm(out, lhsT, rhs, **kw):
        # float32r operands (TF32-like, 4-byte) stream at full PE rate for
        # free dims >= 256; plain fp32 runs at 1/4 rate.  PSUM accum is fp32.
        # Producers write float32r tiles so walrus sees rounded inputs.
        nc.tensor.matmul(out, lhsT, rhs, **kw)

    with tile.TileContext(nc) as tc, ExitStack() as top, \
            nc.allow_low_precision(reason="float32r feeds + bf16 FFN are within the graded tolerance"):
        # Pool lifetimes must nest LIFO per (space, side).  Long-lived pools
        # go on the "right" side, phase-scoped pools on the "left".
        const = top.enter_context(tc.tile_pool(name="const", bufs=1, side="right"))
        identity = const.tile([128, 128], FP32)
        make_identity(nc, identity)
        ones_f = const.tile([1, 512], FP32)
        nc.vector.memset(ones_f, 1.0)
        ones = const.tile([1, 512], F32R)
        nc.vector.tensor_copy(out=ones, in_=ones_f)
        ones_bf = const.tile([1, 1024], BF16)
        nc.vector.memset(ones_bf, 1.0)
        bq_sb = const.tile([1, 128], F32R)
        bk_sb = const.tile([1, 128], F32R)
        bv_sb = const.tile([1, 128], F32R)
        nc.gpsimd.dma_start(out=bq_sb, in_=bq_d)
        nc.gpsimd.dma_start(out=bk_sb, in_=bk_d)
        nc.gpsimd.dma_start(out=bv_sb, in_=bv_d)
        aout = const.tile([128, 4, D], FP32)  # attn_out = LN1(...), lives to the end
        dram = top.enter_context(tc.tile_pool(name="dram", bufs=1, space="DRAM",
                                              side="right"))
        a2a_in = dram.tile([NCORES, 128, 512], F32R)
        a2a_out = dram.tile([NCORES, 128, 512], F32R)

        # persistent attention tensors, two lifetime classes
        pk_stack = ExitStack()           # qT/kT: needed through PASS-Q
        cv_stack = ExitStack()           # v/ctxT: dead after AllToAll send
        pqk = pk_stack.enter_context(tc.tile_pool(name="pqk", bufs=1, side="right"))
        pcv = cv_stack.enter_context(tc.tile_pool(name="pcv", bufs=1, side="right"))
        qT = pqk.tile([128, T], F32R)        # rows 0:64 head0, 64:128 head1
        kT = pqk.tile([128, T], F32R)
        v0 = pcv.tile([128, 16, 130], F32R)  # batch 0: [tok%128, ktile, 65*2]
        v1 = pcv.tile([128, 16, 130], F32R)  # cols 0:64 h0 |64 ones| 65:129 h1 |129 ones
        v_b = [v0, v1]
        ctxT0 = pcv.tile([65, T], F32R)      # head0 unnormalized ctx^T; row 64 = rowsum
        ctxT1 = pcv.tile([65, T], F32R)      # head1
        onecol = const.tile([128, 16, 1], FP32)
        nc.vector.memset(onecol, 1.0)
        for vb in v_b:
            nc.vector.tensor_copy(out=vb[:, :, 64:65], in_=onecol)
            nc.vector.tensor_copy(out=vb[:, :, 129:130], in_=onecol)

        # ---- Phase B: transpose x + QKV projections -------------------
        with ExitStack() as pb:
            pbs = pb.enter_context(tc.tile_pool(name="projs", bufs=1))
            wq_sb = pbs.tile([128, 8, 128], F32R)
            wk_sb = pbs.tile([128, 8, 128], F32R)
            wv_sb = pbs.tile([128, 8, 128], F32R)
            nc.gpsimd.dma_start(out=wq_sb, in_=wq_d.rearrange("(a p) c -> p a c", p=128))
            nc.gpsimd.dma_start(out=wk_sb, in_=wk_d.rearrange("(a p) c -> p a c", p=128))
            nc.gpsimd.dma_start(out=wv_sb, in_=wv_d.rearrange("(a p) c -> p a c", p=128))

            xin_p = pb.enter_context(tc.tile_pool(name="xin", bufs=4))
            xt_p = pb.enter_context(tc.tile_pool(name="xt", bufs=2))
            ps_p = pb.enter_context(tc.tile_pool(name="proj_ps", bufs=1, space="PSUM"))

            for tch in range(8):          # token chunks of 512
                bb = tch // 4
                xT = xt_p.tile([128, 8, 512], F32R)  # [d%128, dslab, tok]
                nc.gpsimd.dma_start(
                    out=xT,
                    in_=xt_d[:, tch * 512:(tch + 1) * 512].rearrange(
                        "(a p) c -> p a c", p=128))
                # q^T / k^T for this chunk: [128(2 heads*dk), 512]
                for w_sb, b_sb, dst in ((wq_sb, bq_sb, qT), (wk_sb, bk_sb, kT)):
                    ps = ps_p.tile([128, 512], FP32, tag="qk", bufs=3)
                    for dsl in range(8):
                        mm(ps, w_sb[:, dsl, :], xT[:, dsl, :],
                           start=(dsl == 0), stop=False)
                    mm(ps, b_sb, ones[:, 0:512], start=False, stop=True)
                    nc.vector.tensor_copy(out=dst[:, tch * 512:(tch + 1) * 512], in_=ps)
                # v^T then PE-transpose to token-major [128 tok, 2*65]
                vps = ps_p.tile([128, 512], FP32, tag="qk", bufs=3)
                for dsl in range(8):
                    mm(vps, wv_sb[:, dsl, :], xT[:, dsl, :],
                       start=(dsl == 0), stop=False)
                mm(vps, bv_sb, ones[:, 0:512], start=False, stop=True)
                vT_sb = xin_p.tile([128, 512], FP32)
                nc.vector.tensor_copy(out=vT_sb, in_=vps)
                vtr = ps_p.tile([128, 4, 128], FP32, tag="v", bufs=2)
                for sub in range(4):
                    nc.tensor.transpose(vtr[:, sub, :],
                                        vT_sb[:, sub * 128:(sub + 1) * 128], identity)
                kt0 = (tch % 4) * 4
                nc.vector.tensor_copy(out=v_b[bb][:, kt0:kt0 + 4, 0:64],
                                      in_=vtr[:, :, 0:64])
                nc.vector.tensor_copy(out=v_b[bb][:, kt0:kt0 + 4, 65:129],
                                      in_=vtr[:, :, 64:128])

        # ---- Phase C1 (PASS-K): ctx^T + softmax denominators ----------
        with ExitStack() as pk:
            e_p = pk.enter_context(tc.tile_pool(name="epool", bufs=1))
            sc_p = pk.enter_context(tc.tile_pool(name="sc_ps", bufs=1, space="PSUM"))
            cx_p = pk.enter_context(tc.tile_pool(name="cx_ps", bufs=1, space="PSUM"))
            for bb in range(2):
                tok0 = bb * S
                for qp in range(2):       # 1024-wide query chunks
                    q0 = tok0 + qp * 1024
                    q_sl = slice(q0, q0 + 1024)
                    cxa = cx_p.tile([65, 1024], FP32, tag="cxa", bufs=1)
                    cxb = cx_p.tile([65, 1024], FP32, tag="cxb", bufs=1)
                    for kt in range(16):  # 128-wide key tiles
                        k_sl = slice(tok0 + kt * 128, tok0 + (kt + 1) * 128)
                        sca = sc_p.tile([128, 1024], FP32, tag="sca", bufs=1)
                        scb = sc_p.tile([128, 1024], FP32, tag="scb", bufs=1)
                        for qh in range(2):
                            qs = slice(q0 + qh * 512, q0 + (qh + 1) * 512)
                            os_ = slice(qh * 512, (qh + 1) * 512)
                            mm(sca[:, os_], kT[0:64, k_sl], qT[0:64, qs],
                               start=True, stop=True)
                            mm(scb[:, os_], kT[64:128, k_sl], qT[64:128, qs],
                               start=True, stop=True)
                        ea = e_p.tile([128, 1024], F32R, tag="ea", bufs=2)
                        eb = e_p.tile([128, 1024], F32R, tag="eb", bufs=2)
                        nc.scalar.activation(ea, sca, AF.Exp, scale=0.125)
                        nc.scalar.activation(eb, scb, AF.Exp, scale=0.125)
                        for qh in range(2):
                            os_ = slice(qh * 512, (qh + 1) * 512)
                            mm(cxa[:, os_], v_b[bb][:, kt, 0:65], ea[:, os_],
                               start=(kt == 0), stop=(kt == 15))
                            mm(cxb[:, os_], v_b[bb][:, kt, 65:130], eb[:, os_],
                               start=(kt == 0), stop=(kt == 15))
                    nc.vector.tensor_copy(out=ctxT0[:, q_sl], in_=cxa)
                    nc.vector.tensor_copy(out=ctxT1[:, q_sl], in_=cxb)

        # ---- Phase D: normalize ctx^T, AllToAll to token-split --------
        # 1/rowsum broadcast via PE outer product (ones x recip-row) straight
        # into PSUM; no DRAM round-trip on the pre-collective critical path.
        with ExitStack() as pd:
            dn_p = pd.enter_context(tc.tile_pool(name="dnorm", bufs=1))
            d_ps = pd.enter_context(tc.tile_pool(name="d_ps", bufs=1, space="PSUM"))
            for hh, ct in ((0, ctxT0), (1, ctxT1)):
                rrow = dn_p.tile([1, T], F32R, name=f"rrow{hh}", tag=f"rrow{hh}")
                nc.vector.tensor_copy(out=rrow, in_=ct[64:65, :])
                nc.vector.reciprocal(out=rrow, in_=rrow)
                for ch in range(8):
                    rp = d_ps.tile([64, 512], FP32, tag="rp", bufs=2)
                    mm(rp, ones[0:1, 0:64], rrow[:, ch * 512:(ch + 1) * 512],
                       start=True, stop=True)
                    nc.vector.tensor_mul(
                        out=ct[0:64, ch * 512:(ch + 1) * 512],
                        in0=ct[0:64, ch * 512:(ch + 1) * 512], in1=rp)
        for j in range(NCORES):
            nc.sync.dma_start(out=a2a_in[j, 0:64, :],
                              in_=ctxT0[0:64, j * 512:(j + 1) * 512])
            nc.sync.dma_start(out=a2a_in[j, 64:128, :],
                              in_=ctxT1[0:64, j * 512:(j + 1) * 512])
        nc.gpsimd.collective_compute(
            "AllToAll", ALU.bypass,
            replica_groups=[list(range(NCORES))],
            ins=[a2a_in.opt()], outs=[a2a_out.opt()])
        cv_stack.close()
        # ---- Phases C2 (PASS-Q) + E (out-proj/LN1), interleaved -------
        # PASS-Q part A is traced first so its matmuls/exps fill the wait
        # for the AllToAll; phase E slots in right as ctxr lands; PASS-Q
        # part B covers the out-proj/LN1 tail.  The attn-output DMA stream
        # then overlaps the whole FFN.
        def layer_norm4(pool, srcs, dsts, g_sb, be_sb):
            # Batched LN over four [128, D] tiles: one Ln + one Exp call so
            # the ACT table set switches at most twice per phase.
            # std' = sqrt(sum c^2/(D-1)) via exp(-0.5*ln(var*D/(D-1)));
            # the reference's +eps=1e-6 on std is dropped (|rel err| ~1e-6).
            mv4 = pool.tile([128, 4, 2], FP32, tag="mv4", bufs=1)
            inv4 = pool.tile([128, 4], FP32, tag="inv4", bufs=1)
            for i, src_t in enumerate(srcs):
                stats = pool.tile([128, 2, 6], FP32, tag="stats", bufs=2)
                nc.vector.bn_stats(out=stats[:, 0, :], in_=src_t[:, 0:512])
                nc.vector.bn_stats(out=stats[:, 1, :], in_=src_t[:, 512:1024])
                nc.vector.bn_aggr(out=mv4[:, i, :], in_=stats)
            nc.scalar.activation(inv4, mv4[:, :, 1], AF.Ln, scale=LN_SCALE)
            nc.scalar.activation(inv4, inv4, AF.Exp, scale=-0.5)
            for i, (src_t, dst) in enumerate(zip(srcs, dsts)):
                nc.vector.tensor_scalar(out=dst, in0=src_t,
                                        scalar1=mv4[:, i, 0:1],
                                        scalar2=inv4[:, i:i + 1],
                                        op0=ALU.subtract, op1=ALU.mult)
                nc.vector.tensor_mul(out=dst, in0=dst, in1=g_sb)
                nc.vector.tensor_add(out=dst, in0=dst, in1=be_sb)

        c2s = ExitStack()   # SBUF pools
        c2p = ExitStack()   # PSUM pools (close before fc2 needs all 8 banks)
        st_p = c2s.enter_context(tc.tile_pool(name="stage", bufs=1))
        sm_p = c2s.enter_context(tc.tile_pool(name="smalls", bufs=1))
        sq_p = c2p.enter_context(tc.tile_pool(name="sq_ps", bufs=1, space="PSUM"))
        e_stack = ExitStack()
        ctxr_p = e_stack.enter_context(tc.tile_pool(name="ctxr", bufs=1))
        wo_p = e_stack.enter_context(tc.tile_pool(name="wo", bufs=1))
        ln_p = e_stack.enter_context(tc.tile_pool(name="ln1", bufs=1))
        ai_p = e_stack.enter_context(tc.tile_pool(name="attn_in", bufs=2))
        wo_ps = e_stack.enter_context(tc.tile_pool(name="wo_ps", bufs=1, space="PSUM"))
        # ctxr: AllToAll result, cast to bf16 on the fly for the Wo matmul
        ctxr = ctxr_p.tile([128, 8, 512], F32R)
        for dsl in range(8):
            nc.gpsimd.dma_start(out=ctxr[:, dsl, :], in_=a2a_out[dsl])

        def pass_q(pairs):
            for bb, qt in pairs:
                tok0 = bb * S
                q_sl = slice(tok0 + qt * 128, tok0 + (qt + 1) * 128)
                stg = [st_p.tile([128, S], FP32, name=f"stg{j}",
                                 tag=f"stg{j}", bufs=2) for j in range(2)]
                parts = sm_p.tile([128, 2, 2], FP32, bufs=2)
                for kp in range(2):   # 1024-wide key chunks
                    for j, base in ((0, slice(0, 64)), (1, slice(64, 128))):
                        sq = sq_p.tile([128, 1024], FP32, name=f"sq{j}",
                                       tag=f"sq{j}", bufs=1)
                        for kh in range(2):
                            ks = slice(tok0 + kp * 1024 + kh * 512,
                                       tok0 + kp * 1024 + (kh + 1) * 512)
                            mm(sq[:, kh * 512:(kh + 1) * 512],
                               qT[base, q_sl], kT[base, ks],
                               start=True, stop=True)
                        nc.scalar.activation(
                            stg[j][:, kp * 1024:(kp + 1) * 1024], sq, AF.Exp,
                            scale=0.125, accum_out=parts[:, j, kp:kp + 1])
                for j in range(2):
                    rq = sm_p.tile([128, 1], FP32, tag="rq", bufs=4)
                    nc.vector.reduce_sum(out=rq, in_=parts[:, j, :], axis=AX.X)
                    nc.vector.reciprocal(out=rq, in_=rq)
                    nc.vector.tensor_scalar_mul(out=stg[j], in0=stg[j], scalar1=rq)
                    nc.sync.dma_start(
                        out=attn_d[bb, j, qt * 128:(qt + 1) * 128, :], in_=stg[j])

        all_qt = [(bb, qt) for bb in range(2) for qt in range(16)]
        pass_q(all_qt[:20])

        # ---- Phase E: out-proj (bf16) + residual + LN1 ----------------
        bo_sb = ln_p.tile([1, D], F32R)
        g1_sb = ln_p.tile([128, D], FP32)
        be1_sb = ln_p.tile([128, D], FP32)
        nc.gpsimd.dma_start(out=bo_sb, in_=bo_d)
        nc.gpsimd.dma_start(out=g1_sb, in_=bcast(g1_d))
        nc.gpsimd.dma_start(out=be1_sb, in_=bcast(be1_d))
        wo_sb = []
        for dsl in range(8):
            w = wo_p.tile([128, D], F32R, name=f"wo{dsl}", tag="wo_slab", bufs=8)
            nc.gpsimd.dma_start(out=w, in_=wo_d[dsl * 128:(dsl + 1) * 128, :])
            wo_sb.append(w)
        a_ins = []
        for sub in range(4):
            a_in = ai_p.tile([128, D], FP32, name=f"a_in{sub}",
                             tag=f"a_in{sub}", bufs=1)
            xc_sb = ai_p.tile([128, D], FP32, tag="xc_sb", bufs=2)
            nc.gpsimd.dma_start(out=xc_sb, in_=xc_d[sub * 128:(sub + 1) * 128, :])
            for oc in range(2):
                ps = wo_ps.tile([128, 512], FP32, tag="wops", bufs=3)
                for dsl in range(8):
                    nc.tensor.matmul(ps, ctxr[:, dsl, sub * 128:(sub + 1) * 128],
                                     wo_sb[dsl][:, oc * 512:(oc + 1) * 512],
                                     start=(dsl == 0), stop=False)
                nc.tensor.matmul(ps, ones[0:1, 0:128],
                                 bo_sb[:, oc * 512:(oc + 1) * 512],
                                 start=False, stop=True)
                nc.vector.tensor_add(out=a_in[:, oc * 512:(oc + 1) * 512],
                                     in0=ps, in1=xc_sb[:, oc * 512:(oc + 1) * 512])
            a_ins.append(a_in)
        layer_norm4(ln_p, a_ins, [aout[:, sub, :] for sub in range(4)],
                    g1_sb, be1_sb)

        pass_q(all_qt[20:])
        e_stack.close()

        # ---- Phase F+G: transpose attn_out, fc1 (relu), all bf16 ------
        fgs = ExitStack()   # SBUF pools, live through fc2
        fgp = ExitStack()   # PSUM pools for transpose + fc1
        at_p = fgs.enter_context(tc.tile_pool(name="at", bufs=1))
        w1_p = fgs.enter_context(tc.tile_pool(name="w1", bufs=3))
        ht_p = fgs.enter_context(tc.tile_pool(name="ht", bufs=1))
        w2_p = fgs.enter_context(tc.tile_pool(name="w2", bufs=4))
        ln2_p = fgs.enter_context(tc.tile_pool(name="ln2", bufs=1))
        enc_p = fgs.enter_context(tc.tile_pool(name="encs", bufs=2))
        g_ps = fgp.enter_context(tc.tile_pool(name="g_ps", bufs=1, space="PSUM"))
        b2_sb = ln2_p.tile([1, D], BF16)
        g2_sb = ln2_p.tile([128, D], FP32)
        be2_sb = ln2_p.tile([128, D], FP32)
        nc.gpsimd.dma_start(out=b2_sb, in_=b2_d)
        nc.gpsimd.dma_start(out=g2_sb, in_=bcast(g2_d))
        nc.gpsimd.dma_start(out=be2_sb, in_=bcast(be2_d))
        aT = at_p.tile([128, 8, 512], BF16)
        for sub in range(4):
            for dg in range(2):
                trq = g_ps.tile([128, 4, 128], FP32, tag="tr2", bufs=1)
                for dj in range(4):
                    dsl = dg * 4 + dj
                    nc.tensor.transpose(
                        trq[:, dj, :], aout[:, sub, dsl * 128:(dsl + 1) * 128],
                        identity)
                nc.vector.tensor_copy(
                    out=aT[:, dg * 4:(dg + 1) * 4, sub * 128:(sub + 1) * 128],
                    in_=trq)
        hT = ht_p.tile([128, 32, 512], BF16)   # [f%128, fslab, tok]
        b1_sb = at_p.tile([1, F], BF16)
        nc.gpsimd.dma_start(out=b1_sb, in_=b1_d)
        for fsl in range(32):
            # stream W1 column-block [1024, 128] as [128 dpart, 8 dslab, 128 f]
            w1c = w1_p.tile([128, 8, 128], BF16, tag="w1_col", bufs=3)
            nc.gpsimd.dma_start(
                out=w1c,
                in_=w1_d[:, fsl * 128:(fsl + 1) * 128].rearrange(
                    "(a p) c -> p a c", p=128))
            ps = g_ps.tile([128, 512], FP32, tag="hps", bufs=3)
            for dsl in range(8):
                nc.tensor.matmul(ps, w1c[:, dsl, :], aT[:, dsl, :],
                                 start=(dsl == 0), stop=False)
            nc.tensor.matmul(ps, b1_sb[:, fsl * 128:(fsl + 1) * 128],
                             ones_bf[:, 0:512], start=False, stop=True)
            nc.vector.tensor_scalar_max(out=hT[:, fsl, :], in0=ps, scalar1=0.0)
        fgp.close()
        c2p.close()

        # ---- Phase H+I: fc2 + residual + LN2, write enc ---------------
        # fc2 in two half-column passes of 4 PSUM banks each, so pass A can
        # start as soon as fc1's banks free (PASS-Q's banks may still be live)
        php = ExitStack()
        f_ps = php.enter_context(tc.tile_pool(name="f_ps", bufs=1, space="PSUM"))
        groups = {}
        for oc in range(2):
            for sub in range(4):
                groups[(sub, oc)] = f_ps.tile(
                    [128, 512], FP32, name=f"ff{sub}{oc}",
                    tag=f"ff{oc}", bufs=4)
            for fsl in range(32):
                w2s = w2_p.tile([128, 512], BF16, name=f"w2s{oc}",
                                tag="w2_slab", bufs=4)
                nc.gpsimd.dma_start(
                    out=w2s,
                    in_=w2_d[fsl * 128:(fsl + 1) * 128, oc * 512:(oc + 1) * 512])
                for sub in range(4):
                    nc.tensor.matmul(groups[(sub, oc)],
                                     hT[:, fsl, sub * 128:(sub + 1) * 128],
                                     w2s,
                                     start=(fsl == 0), stop=False)
        enc_pres, enc_outs = [], []
        for sub in range(4):
            enc_pre = enc_p.tile([128, D], FP32, name=f"encpre{sub}",
                                 tag=f"encpre{sub}", bufs=1)
            enc_out = enc_p.tile([128, D], FP32, name=f"encout{sub}",
                                 tag=f"encout{sub}", bufs=1)
            for oc in range(2):
                nc.tensor.matmul(groups[(sub, oc)],
                                 ones_bf[:, 0:128],
                                 b2_sb[:, oc * 512:(oc + 1) * 512],
                                 start=False, stop=True)
                nc.vector.tensor_add(out=enc_pre[:, oc * 512:(oc + 1) * 512],
                                     in0=groups[(sub, oc)],
                                     in1=aout[:, sub, oc * 512:(oc + 1) * 512])
            enc_pres.append(enc_pre)
            enc_outs.append(enc_out)
        layer_norm4(ln2_p, enc_pres, enc_outs, g2_sb, be2_sb)
        for sub in range(4):
            nc.sync.dma_start(out=enc_d[sub * 128:(sub + 1) * 128, :],
                              in_=enc_outs[sub])
        php.close()
        fgs.close()
        c2s.close()
        pk_stack.close()

    nc.compile()
    return nc


def make_in_maps(inputs):
    inp = {k: np.asarray(v) for k, v in inputs.items()}
    x = inp["x"].reshape(T, D).astype(np.float32)
    xt = np.ascontiguousarray(x.T)
    r1 = lambda a: np.ascontiguousarray(a.astype(np.float32).reshape(1, -1))
    rb = lambda a: np.ascontiguousarray(a.reshape(1, -1).astype(ml_dtypes.bfloat16))
    w1 = np.ascontiguousarray(inp["W1"].astype(ml_dtypes.bfloat16))
    w2 = np.ascontiguousarray(inp["W2"].astype(ml_dtypes.bfloat16))
    in_maps = []
    for c in range(NCORES):
        cs = slice(128 * c, 128 * (c + 1))
        in_maps.append({
            "xt": xt,
            "xc": np.ascontiguousarray(x[TC * c: TC * (c + 1)]),
            "wq": np.ascontiguousarray(inp["Wq"].astype(np.float32)[:, cs]),
            "wk": np.ascontiguousarray(inp["Wk"].astype(np.float32)[:, cs]),
            "wv": np.ascontiguousarray(inp["Wv"].astype(np.float32)[:, cs]),
            "bq": r1(inp["bq"][cs]),
            "bk": r1(inp["bk"][cs]),
            "bv": r1(inp["bv"][cs]),
            "wo": np.ascontiguousarray(inp["Wo"].astype(np.float32)),
            "bo": r1(inp["bo"]),
            "w1": w1,
            "b1": rb(inp["b1"]),
            "w2": w2,
            "b2": rb(inp["b2"]),
            "g1": r1(inp["alpha1"]),
            "be1": r1(inp["beta1"]),
            "g2": r1(inp["alpha2"]),
            "be2": r1(inp["beta2"]),
        })
    return in_maps


def assemble(results):
    attn = np.empty((B, H, S, S), np.float32)
    enc = np.empty((T, D), np.float32)
    for c in range(NCORES):
        attn[:, 2 * c: 2 * c + 2] = results[c]["attn_o"]
        enc[TC * c: TC * (c + 1)] = results[c]["enc_o"]
    return enc.reshape(B, S, D), attn


def kernel(**inputs):
    nc = build_program()
    in_maps = make_in_maps(inputs)
    res = run_bass_kernel_spmd(nc, in_maps, list(range(NCORES))).results
    return assemble(res)
